# revision 11
# baseline (speedup 1.0000x reference)
"""Trainium2 Bass kernel for nn_MHA_9603546874182.

Causal MHA: qkv proj + rope(32) + causal attention + out proj.
B=4, T=1024, C=2048, H=32, hd=64.

Sharding: 8-way tensor parallel over heads (4 heads / core).
Each core computes qkv for its 4 heads (column-parallel), rope,
causal attention, and a row-parallel partial of the output
projection. Host sums the 8 bf16 partials (+ bias) in fp32.

v3 layout (all SBUF tensors bf16, PSUM fp32):
  phase 1 (per batch, per 128-token tile): qkv = x^T.T @ Wshard
      (token-major, psum), DVE bias-add -> bf16 sbuf, rope on q|k
      jointly, PE-transpose q|k into one [128,512] psum tile, one
      DVE copy -> QKT [128, m8, 4, 128]; v (+ones col) token-major.
  phase 2 (per head): per key-tile st, scores^T for the causally
      needed query range [128*st, T) only; exp on ACT (psum -> bf16
      sbuf), 0/1 diag-mask multiply after exp, AV accumulation
      (lhsT=[V|1]) giving ctx^T and the softmax row-sum in psum
      [65, 512] per query half.  Normalize with
      reciprocal_approx_fast + gpsimd broadcast + DVE multiply.
  phase 3: partial out = ctx^T.T @ W2shard, psum -> bf16 (DVE),
      one 4KB-per-partition DMA per token tile.

Batches are software-pipelined (big tensors triple-buffered) so the
PE stream stays dense and HAM stays at 8/8.
"""

import numpy as np
import ml_dtypes

B, T, C, H = 4, 1024, 2048, 32
HD = C // H          # 64
NCORES = 8
HPC = H // NCORES    # 4 heads per core
SC = HPC * HD        # 256 shard channels
NTOK = B * T         # 4096
KT16 = C // 128      # 16 k tiles
MT = NTOK // 128     # 32 token tiles
MPB = T // 128       # 8 token tiles per batch
ROT = 32
VW = 66              # V row width: 64 data + 1 ones + 1 pad (4B align)

_CACHE = {}


def _build_nc():
    import concourse.bass as bass
    import concourse.mybir as mybir
    import concourse.tile as tile
    from concourse import bacc
    from concourse.masks import make_identity

    f32 = mybir.dt.float32
    bf16 = mybir.dt.bfloat16

    nc = bacc.Bacc("TRN2")

    xt_d = nc.dram_tensor("xt", [128, MT, KT16, 128], bf16, kind="ExternalInput")
    wq_d = nc.dram_tensor("wq", [128, KT16, 3 * SC], bf16, kind="ExternalInput")
    br_d = nc.dram_tensor("br", [128, 3 * SC], f32, kind="ExternalInput")
    c1_d = nc.dram_tensor("c1", [128, MPB, 2 * SC], bf16, kind="ExternalInput")
    c2_d = nc.dram_tensor("c2", [128, MPB, 2 * SC], bf16, kind="ExternalInput")
    dg_d = nc.dram_tensor("dg", [128, 128], bf16, kind="ExternalInput")
    w2_d = nc.dram_tensor("w2", [128, 2, C], bf16, kind="ExternalInput")
    out_d = nc.dram_tensor("out", [MT, 128, C], bf16, kind="ExternalOutput")

    EXP = mybir.ActivationFunctionType.Exp

    with tile.TileContext(nc) as tc:
        with (
            tc.tile_pool(name="const", bufs=1) as const,
            tc.tile_pool(name="xp", bufs=3) as xp,
            tc.tile_pool(name="qkp", bufs=3) as qkp,
            tc.tile_pool(name="rtp", bufs=2) as rtp,
            tc.tile_pool(name="bigp", bufs=3) as bigp,
            tc.tile_pool(name="ptp", bufs=4) as ptp,
            tc.tile_pool(name="rsp", bufs=2) as rsp,
            tc.tile_pool(name="outp", bufs=2) as outp,
            tc.tile_pool(name="ps1", bufs=2, space="PSUM") as ps1,
            tc.tile_pool(name="psc", bufs=3, space="PSUM") as psc,
            tc.tile_pool(name="pcp", bufs=1, space="PSUM") as pcp,
        ):
            # small consts on the sync queue (first-MM dependencies first)
            br = const.tile([128, 3 * SC], f32)
            nc.sync.dma_start(br[:], br_d[:])
            dg = const.tile([128, 128], bf16)
            nc.sync.dma_start(dg[:], dg_d[:])
            ident = const.tile([128, 128], bf16)
            make_identity(nc, ident)
            # latency-tolerant consts on the scalar hwdge queue
            c1 = const.tile([128, MPB, 2 * SC], bf16)
            nc.scalar.dma_start(c1[:], c1_d[:])
            c2 = const.tile([128, MPB, 2 * SC], bf16)
            nc.scalar.dma_start(c2[:], c2_d[:])
            w2 = const.tile([128, 2, C], bf16)
            nc.scalar.dma_start(w2[:], w2_d[:])
            wq = const.tile([128, KT16, 3 * SC], bf16)

            brv = br[:, 2 * SC:3 * SC].rearrange("p (h d) -> p h d", h=HPC)

            for b in range(B):
                # QKT[:, m8, s, t]: s in 0,1 = q chans, 2,3 = k chans
                QKT = bigp.tile([128, MPB, 4, 128], bf16, tag="qkt")
                Vp = bigp.tile([128, MPB, HPC, VW], bf16, tag="vp")
                ctxT = bigp.tile([128, 2, T], bf16, tag="ct")
                nc.vector.memset(Vp[:, :, :, 64:65], 1.0)

                # ---- phase 1: qkv + rope + transpose ----
                for m8 in range(MPB):
                    m = b * MPB + m8
                    xt = xp.tile([128, KT16, 128], bf16)
                    nc.sync.dma_start(xt[:], xt_d[:, m, :, :])
                    pqkv = ps1.tile([128, 768], f32, tag="qkv")
                    for k in range(KT16):
                        if b == 0 and m8 == 0:
                            nc.sync.dma_start(wq[:, k, :], wq_d[:, k, :])
                        nc.tensor.matmul(
                            pqkv[:, 0:512], xt[:, k, :], wq[:, k, 0:512],
                            start=(k == 0), stop=(k == KT16 - 1))
                        nc.tensor.matmul(
                            pqkv[:, 512:768], xt[:, k, :], wq[:, k, 512:768],
                            start=(k == 0), stop=(k == KT16 - 1))
                    qk = qkp.tile([128, 512], bf16)
                    nc.vector.tensor_add(qk[:], pqkv[:, 0:512], br[:, 0:512])
                    nc.vector.tensor_add(
                        Vp[:, m8, :, 0:HD],
                        pqkv[:, 512:768].rearrange("p (h d) -> p h d", h=HPC),
                        brv)
                    # rope on q|k jointly (8 head-sections of 64)
                    qk8 = qk.rearrange("p (g d) -> p g d", d=HD)
                    c18 = c1[:, m8, :]
                    c28 = c2[:, m8, :].rearrange("p (g d) -> p g d", d=HD)
                    rt = rtp.tile([128, 8, ROT], bf16)
                    nc.vector.tensor_mul(
                        rt[:, :, 0:16], qk8[:, :, 16:32], c28[:, :, 0:16])
                    nc.vector.tensor_mul(
                        rt[:, :, 16:32], qk8[:, :, 0:16], c28[:, :, 16:32])
                    nc.vector.tensor_mul(qk[:], qk[:], c18)
                    nc.vector.tensor_add(
                        qk8[:, :, 0:ROT], qk8[:, :, 0:ROT], rt[:])
                    # PE-transpose all 4 128-chan blocks into one psum tile
                    tp = psc.tile([128, 512], bf16, tag="sc")
                    for s in range(4):
                        nc.tensor.transpose(
                            tp[:, s * 128:(s + 1) * 128],
                            qk[:, s * 128:(s + 1) * 128], ident)
                    nc.vector.tensor_copy(QKT[:, m8, :, :], tp[:])

                # ---- phase 2: attention ----
                def norm(pc, p0, ci, qh):
                    ss = rsp.tile([1, 512], f32, tag="ss")
                    nc.scalar.copy(ss[:], pc[64:65, :])
                    rs = rsp.tile([1, 512], f32, tag="rs")
                    nc.vector.reciprocal_approx_fast(rs[:], ss[:])
                    rsb = rsp.tile([64, 512], f32, tag="rsb")
                    nc.gpsimd.partition_broadcast(rsb[:], rs[:])
                    nc.vector.tensor_mul(
                        ctxT[p0:p0 + 64, ci, qh * 512:(qh + 1) * 512],
                        pc[0:64, :], rsb[:])

                for h in range(HPC):
                    ci = h // 2
                    p0 = (h % 2) * 64
                    kth = QKT[p0:p0 + 64, :, 2 + ci, :]   # [64, m8, 128]
                    qth = QKT[p0:p0 + 64, :, ci, :]       # [64, m8, 128]
                    # pass A: query half 0, key tiles 0..3
                    pc0 = pcp.tile([65, 512], f32, tag="pc")
                    for st in range(4):
                        lo = 128 * st
                        scA = psc.tile([128, 512], f32, tag="sc")
                        nc.tensor.matmul(
                            scA[:, lo:512], kth[:, st, :], qth[:, st:4, :],
                            start=True, stop=True)
                        ptA = ptp.tile([128, 512], bf16, tag="pt")
                        nc.scalar.activation(
                            ptA[:, lo:512], scA[:, lo:512], EXP)
                        nc.vector.tensor_mul(
                            ptA[:, lo:lo + 128], ptA[:, lo:lo + 128], dg[:])
                        nc.tensor.matmul(
                            pc0[:, lo:512], Vp[:, st, h, 0:65], ptA[:, lo:512],
                            start=(st == 0), stop=(st == 3),
                            skip_group_check=True)
                    norm(pc0, p0, ci, 0)
                    # pass B: query half 1, key tiles 0..7
                    pc1 = pcp.tile([65, 512], f32, tag="pc")
                    for st in range(MPB):
                        lo = 128 * st
                        lb = max(0, lo - 512)
                        scB = psc.tile([128, 512], f32, tag="sc")
                        nc.tensor.matmul(
                            scB[:, lb:512], kth[:, st, :],
                            qth[:, max(st, 4):8, :], start=True, stop=True)
                        ptB = ptp.tile([128, 512], bf16, tag="pt")
                        nc.scalar.activation(ptB[:, lb:512], scB[:, lb:512], EXP)
                        if st >= 4:
                            nc.vector.tensor_mul(
                                ptB[:, lb:lb + 128], ptB[:, lb:lb + 128], dg[:])
                        nc.tensor.matmul(
                            pc1[:, lb:512], Vp[:, st, h, 0:65], ptB[:, lb:512],
                            start=(st == 0), stop=(st == MPB - 1),
                            skip_group_check=True)
                    norm(pc1, p0, ci, 1)

                # ---- phase 3: out projection partial ----
                for m8 in range(MPB):
                    m = b * MPB + m8
                    ot = outp.tile([128, C], bf16)
                    for n in range(4):
                        po = psc.tile([128, 512], f32, tag="sc")
                        for j in range(2):
                            nc.tensor.matmul(
                                po[:], ctxT[:, j, m8 * 128:(m8 + 1) * 128],
                                w2[:, j, n * 512:(n + 1) * 512],
                                start=(j == 0), stop=(j == 1))
                        nc.vector.tensor_copy(ot[:, n * 512:(n + 1) * 512], po[:])
                    nc.sync.dma_start(out_d[m, :, :], ot[:])

    nc.finalize()
    return nc


def _host_prep(x, rope, Wqkv_w, Wqkv_b, out_w):
    """Build per-core input maps (bf16 partition-first layouts)."""
    bf = ml_dtypes.bfloat16
    xf = np.ascontiguousarray(x.reshape(NTOK, C)).astype(np.float32)
    # xt[p, m, k, t] = x[m*128+t, k*128+p]
    xt = np.ascontiguousarray(
        xf.reshape(MT, 128, KT16, 128).transpose(3, 0, 2, 1).astype(bf))

    # rope tables (position within a batch: t = 0..1023), q|k joint: 8
    # sections of 64 channels
    cos = rope[:, :, 0].astype(np.float32)   # [T, 16]
    sin = rope[:, :, 1].astype(np.float32)
    C1h = np.ones((T, HD), np.float32)
    C1h[:, 0:16] = cos
    C1h[:, 16:32] = cos
    C2h = np.zeros((T, HD), np.float32)
    C2h[:, 0:16] = -sin
    C2h[:, 16:32] = sin
    C1 = np.tile(C1h, (1, 2 * HPC))          # [T, 512]
    C2 = np.tile(C2h, (1, 2 * HPC))
    c1 = np.ascontiguousarray(
        C1.reshape(MPB, 128, 2 * SC).transpose(1, 0, 2).astype(bf))
    c2 = np.ascontiguousarray(
        C2.reshape(MPB, 128, 2 * SC).transpose(1, 0, 2).astype(bf))

    # diag-tile 0/1 mask: dg[p, y] = 1 if query offset y >= key offset p
    yy = np.arange(128)[None, :]
    pp = np.arange(128)[:, None]
    dgm = (yy >= pp).astype(bf)

    scale = np.float32(1.0 / np.sqrt(HD))
    in_maps = []
    for g in range(NCORES):
        hs = g * SC
        Wq = Wqkv_w[hs:hs + SC, :].astype(np.float32) * scale
        Wk = Wqkv_w[C + hs:C + hs + SC, :].astype(np.float32)
        Wv = Wqkv_w[2 * C + hs:2 * C + hs + SC, :].astype(np.float32)
        Wsh = np.concatenate([Wq, Wk, Wv], axis=0)          # [768, 2048]
        # wq[p, k, j] = Wsh[j, k*128+p]
        wqa = np.ascontiguousarray(
            Wsh.T.reshape(KT16, 128, 3 * SC).transpose(1, 0, 2).astype(bf))
        bq = Wqkv_b[hs:hs + SC].astype(np.float32) * scale
        bk = Wqkv_b[C + hs:C + hs + SC].astype(np.float32)
        bv = Wqkv_b[2 * C + hs:2 * C + hs + SC].astype(np.float32)
        bsh = np.concatenate([bq, bk, bv])
        bra = np.ascontiguousarray(np.broadcast_to(bsh, (128, 3 * SC))).astype(
            np.float32)
        # w2[p, j, o] = out_w[o, g*256 + j*128 + p]
        w2a = np.ascontiguousarray(
            out_w[:, hs:hs + SC].astype(np.float32).T.reshape(
                2, 128, C).transpose(1, 0, 2).astype(bf))
        in_maps.append({
            "xt": xt, "wq": wqa, "br": bra, "c1": c1, "c2": c2,
            "dg": dgm, "w2": w2a,
        })
    return in_maps


def kernel(x, mask, index, rope, Wqkv_w, Wqkv_b, out_w, out_b,
           k_cache, v_cache):
    from concourse.bass_utils import run_bass_kernel_spmd

    x = np.asarray(x)
    rope = np.asarray(rope)
    Wqkv_w = np.asarray(Wqkv_w)
    Wqkv_b = np.asarray(Wqkv_b)
    out_w = np.asarray(out_w)
    out_b = np.asarray(out_b)

    if "nc" not in _CACHE:
        _CACHE["nc"] = _build_nc()
    nc = _CACHE["nc"]

    in_maps = _host_prep(x, rope, Wqkv_w, Wqkv_b, out_w)
    res = run_bass_kernel_spmd(nc, in_maps, core_ids=list(range(NCORES)))

    acc = np.zeros((NTOK, C), np.float32)
    for g in range(NCORES):
        acc += res.results[g]["out"].reshape(NTOK, C).astype(np.float32)
    acc += out_b.astype(np.float32)
    return acc.reshape(B, T, C)
